# revision 24
# baseline (speedup 1.0000x reference)
"""Segment-softmax (GAT attention stage 4) Trainium2 kernel, 8 NeuronCores.

alpha_i = exp(e_i) / (sum_{j: tgt_j = tgt_i} exp(e_j) + 1e-16)

Strategy (node-parallel sharding, degree-sorted variable-width packing):
  - The host stable-sorts edges by target node (a pure data-layout
    permutation, inverted after the device run) and shards NODES across the
    8 cores (12500 nodes each) -> each core owns complete segments, so no
    cross-core reduction is needed.
  - Within each core, nodes are ordered by degree (ascending); blocks of
    128 consecutive nodes map to the 128 SBUF partitions.  Blocks are
    grouped into chunks, and every node row in a chunk is padded to the
    chunk's max degree W (rounded up to 8).  Because degrees are sorted,
    total padding is only a few % over the raw edge count.  The chunk plan
    (nb, W) is derived from the input's degree histogram at first call and
    shared across all cores (max over cores per block index).
  - Device work per chunk (one instruction each, all 2x-eligible shapes):
      ACT:  X = exp(E)                       (fp16, padding -100 -> 0)
      DVE:  Hf = X[:,:,0:W/2] + X[:,:,W/2:W] (f16+f16, 2x mode)
      DVE:  S[p,b] = reduce_add(Hf)          (fp32)
      DVE:  R = recip_approx(S)              (custom DVE op)
      DVE:  R16d = dup(min(R*8192, 6e4))     (pair-duplicated fp16 scalars)
      DVE:  A = X * R16d-pairs               (f16 2x via [.., W/2, 2] view)
  - Loads ride the sync+scalar HWDGE rings (alternating); stores ride the
    gpsimd SWDGE ring except the last (latency-critical) one.
  - Host divides by 8192 (exact) while scattering back to original order.
  All arithmetic (exp, segment sums, reciprocal, normalize) runs on device;
  the host only sorts/pads/permutes layouts.
"""
import sys

sys.path.insert(0, "/opt/trn_rl_repo")

import numpy as np
import concourse.bacc as bacc
import concourse.mybir as mybir
import concourse.tile as tile
from concourse import bass_utils

P = 128
N_CORES = 8
NUM_EDGES = 6_400_000
NUM_NODES = 100_000
NPC = NUM_NODES // N_CORES          # 12500 nodes per core
NBLK = (NPC + P - 1) // P           # 98 node blocks per core
SCALE = 8192.0                      # keeps alpha*SCALE in fp16 normal range
FIRST_FD = 512                      # small first chunk: start compute early
TARGET_FD = 1952                    # mid chunks
LAST_FD = 512                       # small last chunk: drain quickly
CAP_FD = 2304
MAXNB = 64

f16, f32 = mybir.dt.float16, mybir.dt.float32
_cache = {}


def make_plan(counts):
    """Chunk plan [(nb, W), ...] covering the NBLK degree-sorted blocks."""
    deg_sorted = np.sort(counts.reshape(N_CORES, NPC), axis=1)
    pad = NBLK * P - NPC
    deg_sorted = np.pad(deg_sorted, ((0, 0), (0, pad)))
    blockmax = deg_sorted.reshape(N_CORES, NBLK, P).max(axis=2).max(axis=0)
    blockmax = np.maximum(blockmax, 1)
    wof = ((blockmax + 7) // 8) * 8          # per-block width if chunk ended
    plan = []
    b = 0
    while b < NBLK:
        tgt = FIRST_FD if not plan else TARGET_FD
        nb = 1
        while b + nb < NBLK and (nb + 1) * wof[b + nb] <= tgt:
            nb += 1
        plan.append((int(nb), int(wof[b + nb - 1])))
        b += nb
    # carve a small tail chunk so the final store is short
    if len(plan) > 1 and plan[-1][0] * plan[-1][1] > LAST_FD:
        nb, W = plan.pop()
        nb_tail = max(1, LAST_FD // W)
        if nb > nb_tail:
            plan.append((nb - nb_tail, W))
        plan.append((min(nb, nb_tail), W))
    return tuple(plan)


def plan_layout(plan):
    """Per-block (colbase, width) arrays and chunk offsets."""
    W_blk = np.empty(NBLK, dtype=np.int64)
    base_blk = np.empty(NBLK, dtype=np.int64)
    chunk_off = []
    o = 0
    b = 0
    for (nb, W) in plan:
        chunk_off.append(o)
        for i in range(nb):
            W_blk[b + i] = W
            base_blk[b + i] = o + i * W
        o += nb * W
        b += nb
    assert b == NBLK
    return W_blk, base_blk, chunk_off, o


MM_W = 8                            # PE partial-sum sub-slice width


def _build(plan):
    W_blk, base_blk, chunk_off, FD = plan_layout(plan)
    nc = bacc.Bacc("TRN2", target_bir_lowering=False, debug=False,
                   enable_asserts=False)
    d_E = nc.dram_tensor("E", [P, FD], f16, kind="ExternalInput")
    d_I = nc.dram_tensor("ident", [P, P], f16, kind="ExternalInput")
    d_A = nc.dram_tensor("alpha", [P, FD], f16, kind="ExternalOutput")
    OP = mybir.AluOpType
    Exp = mybir.ActivationFunctionType.Exp

    with tile.TileContext(nc) as tc:
        with (
            tc.tile_pool(name="const", bufs=1) as cpool,
            tc.tile_pool(name="io", bufs=3) as iopool,
            tc.tile_pool(name="sm", bufs=3) as spool,
            tc.tile_pool(name="ps", bufs=2, space="PSUM") as ppool,
        ):
            ident = cpool.tile([P, P], f16)
            nc.scalar.dma_start(out=ident[:], in_=d_I[:])
            for ci, (nb, W) in enumerate(plan):
                o_lo = chunk_off[ci]
                fdc = nb * W
                hW = W // 2
                assert nb <= MAXNB and fdc <= CAP_FD
                E16 = iopool.tile([P, CAP_FD], f16, tag="E16")
                nc.sync.dma_start(out=E16[:, 0:fdc],
                                  in_=d_E[:, o_lo:o_lo + fdc])
                X16 = iopool.tile([P, CAP_FD], f16, tag="X16")
                nc.scalar.activation(X16[:, 0:fdc], E16[:, 0:fdc], Exp)
                # segment partial sums on the (otherwise idle) PE: accumulate
                # identity-matmuls of MM_W-wide sub-slices into PSUM, so the
                # vector engine only reduces nb*MM_W elements
                v = X16[:, 0:fdc].rearrange("p (n d) -> p n d", d=W)
                PS = ppool.tile([P, MAXNB * MM_W], f32, space="PSUM",
                                tag="PS")
                nmm = W // MM_W
                for j in range(nmm):
                    nc.tensor.matmul(out=PS[:, 0:nb * MM_W],
                                     lhsT=ident[:],
                                     rhs=v[:, :, MM_W * j:MM_W * (j + 1)],
                                     start=(j == 0), stop=(j == nmm - 1))
                S = spool.tile([P, MAXNB], f32, tag="S")
                nc.vector.tensor_reduce(
                    out=S[:, 0:nb],
                    in_=PS[:, 0:nb * MM_W].rearrange("p (n d) -> p n d",
                                                     d=MM_W),
                    axis=mybir.AxisListType.X, op=OP.add)
                R = spool.tile([P, MAXNB], f32, tag="R")
                nc.vector.reciprocal_approx_fast(out=R[:, 0:nb],
                                                 in_=S[:, 0:nb])
                # fused scale+clamp+pair-duplicate: R16d[p,2b+t] =
                # min(R[p,b]*SCALE, 6e4); duplicated pairs let the broadcast
                # multiply read unit-stride f16 (2x mode)
                R16d = spool.tile([P, 2 * MAXNB], f16, tag="R16d")
                nc.vector.tensor_scalar(
                    out=R16d[:, 0:2 * nb].rearrange("p (n t) -> p n t", t=2),
                    in0=R[:, 0:nb].unsqueeze(2).broadcast_to([P, nb, 2]),
                    scalar1=SCALE, scalar2=60000.0,
                    op0=OP.mult, op1=OP.min)
                A16 = iopool.tile([P, CAP_FD], f16, tag="A16")
                xv = X16[:, 0:fdc].rearrange("p (n h t) -> p n h t",
                                             h=hW, t=2)
                av = A16[:, 0:fdc].rearrange("p (n h t) -> p n h t",
                                             h=hW, t=2)
                rb = R16d[:, 0:2 * nb].rearrange(
                    "p (n t) -> p n t", t=2).unsqueeze(2).broadcast_to(
                    [P, nb, hW, 2])
                nc.vector.tensor_tensor(out=av, in0=xv, in1=rb, op=OP.mult)
                if ci == len(plan) - 1:
                    nc.sync.dma_start(out=d_A[:, o_lo:o_lo + fdc],
                                      in_=A16[:, 0:fdc])
                else:
                    nc.gpsimd.dma_start(out=d_A[:, o_lo:o_lo + fdc],
                                        in_=A16[:, 0:fdc])
    nc.compile()
    return nc


def _get_neff(plan):
    if plan not in _cache:
        _cache[plan] = _build(plan)
    return _cache[plan]


def prep_inputs(e, edge_index):
    """Sort edges by target node, degree-sort nodes, chunk-width padding."""
    e = np.asarray(e, dtype=np.float32).reshape(-1)
    t = np.asarray(edge_index)[1].astype(np.int64)
    counts = np.bincount(t, minlength=NUM_NODES)
    plan = make_plan(counts)
    W_blk, base_blk, chunk_off, FD = plan_layout(plan)
    # node -> rank within its core under ascending-degree order
    order = np.argsort(counts.reshape(N_CORES, NPC), axis=1, kind="stable")
    m_of = np.empty((N_CORES, NPC), dtype=np.int64)
    ar = np.arange(NPC, dtype=np.int64)
    for c in range(N_CORES):
        m_of[c, order[c]] = ar
    m = m_of.reshape(-1)                    # global node -> rank in core
    p_of = m % P
    colbase = base_blk[m // P]              # start column per node
    # per-edge destination in the padded layout
    perm = np.argsort(t, kind="stable")
    t_s = t[perm]
    starts = np.zeros(NUM_NODES + 1, dtype=np.int64)
    np.cumsum(counts, out=starts[1:])
    rank = np.arange(NUM_EDGES, dtype=np.int64) - starts[t_s]
    c_e = t_s // NPC
    flat = (c_e * P + p_of[t_s]) * FD + colbase[t_s] + rank
    E = np.full(N_CORES * P * FD, -100.0, dtype=np.float16)
    E[flat] = e[perm].astype(np.float16)
    return E.reshape(N_CORES, P, FD), flat, perm, plan


def make_in_maps(E):
    ident = np.eye(P, dtype=np.float16)
    return [{"E": E[c], "ident": ident} for c in range(N_CORES)]


def kernel(e, edge_index, num_nodes):
    assert int(num_nodes) == NUM_NODES
    E, flat, perm, plan = prep_inputs(e, edge_index)
    nc = _get_neff(plan)
    in_maps = make_in_maps(E)
    res = bass_utils.run_bass_kernel_spmd(nc, in_maps,
                                          core_ids=list(range(N_CORES)))
    A = np.stack([np.asarray(res.results[c]["alpha"])
                  for c in range(N_CORES)])
    alpha_sorted = A.reshape(-1)[flat].astype(np.float32) * np.float32(1.0 / SCALE)
    out = np.empty(NUM_EDGES, dtype=np.float32)
    out[perm] = alpha_sorted
    return out
